# revision 15
# baseline (speedup 1.0000x reference)
"""Trainium2 Bass kernel for nn_ContextEncoder (GRU feature encoder + DenseGAT readout).

Contract: kernel(**inputs) takes the FULL unsharded inputs (numpy, as produced
by setup_inputs) and returns the FULL output [B, CD] float32.

Strategy: data-parallel over the batch axis B across 8 NeuronCores. Each core
processes 16 batches = 2048 (batch, node) rows:
  - feature pipeline (speed + turn-angle) on device
  - 127-step GRU (hidden 128) with bf16 matmuls and fp32 PSUM accumulation
  - dense-GAT readout reduced analytically to small matmuls (only node 0 of
    the attention output is needed, and the per-head linear map commutes with
    the attention-weighted sum).
"""

import sys

sys.path.insert(0, "/opt/trn_rl_repo")

import numpy as np
import ml_dtypes

import concourse.bass as bass
import concourse.bacc as bacc
import concourse.mybir as mybir
import concourse.tile as tile
from concourse.bass_utils import run_bass_kernel_spmd

F32 = mybir.dt.float32
BF16 = mybir.dt.bfloat16
AF = mybir.ActivationFunctionType
ALU = mybir.AluOpType
AX = mybir.AxisListType

N_CORES = 8
B, N, L, HID, CD, HEADS = 128, 128, 128, 128, 128, 4
T = L - 1  # 127 GRU steps
BC = B // N_CORES  # batches per core = 16
R = BC * N  # rows per core = 2048
EPS = 1e-6
NEG_SLOPE = 0.2

# Abramowitz & Stegun 4.4.45: arccos(x) ~= sqrt(1-x) * poly(x), 0<=x<=1,
# |err| <= 6.7e-5 rad.
AC0, AC1, AC2, AC3 = 1.5707288, -0.2121144, 0.0742610, -0.0187293

NSTREAM = 4
SC = R // NSTREAM  # 512 rows per stream chunk


def _build_program(repeats=1):
    nc = bacc.Bacc("TRN2", target_bir_lowering=False, debug=False,
                   num_devices=N_CORES)

    # Per-core inputs (already sharded/laid out by the host wrapper).
    xr_d = nc.dram_tensor("xr", [R, 2 * L], F32, kind="ExternalInput")
    whhT_d = nc.dram_tensor("whhT", [HID, 3 * HID], BF16, kind="ExternalInput")
    wih_d = nc.dram_tensor("wih_aug", [3, 3 * HID], BF16, kind="ExternalInput")
    bhhn_d = nc.dram_tensor("bhh_n", [1, HID], BF16, kind="ExternalInput")
    ident_d = nc.dram_tensor("ident", [128, 128], BF16, kind="ExternalInput")
    uwd_d = nc.dram_tensor("uwd", [HID, 2 * HEADS], BF16, kind="ExternalInput")
    wgT_d = nc.dram_tensor("wgT", [HID, HEADS * CD], BF16, kind="ExternalInput")
    gbias_d = nc.dram_tensor("gbias", [1, CD], BF16, kind="ExternalInput")
    out_d = nc.dram_tensor("out", [BC, CD], F32, kind="ExternalOutput")

    NT = R // 128  # 16 row tiles
    with tile.TileContext(nc) as tc:
        with (
            tc.tile_pool(name="dram", bufs=1, space="DRAM") as dpool,
            tc.tile_pool(name="const", bufs=1) as cpool,
        ):
            f3 = dpool.tile([T, 3, R], BF16)  # per-step rhs rows (v, ang, 1)
            ident = cpool.tile([128, 128], BF16, tag="ident")
            nc.sync.dma_start(ident[:], ident_d.ap())
            ones = cpool.tile([1, R], BF16, tag="ones")
            nc.vector.memset(ones[:], 1.0)
            for _ in range(repeats):
                _build_features(nc, tc, xr_d, f3, NT, ident)
                _build_gru_gat(nc, tc, f3, whhT_d, wih_d, bhhn_d, ident,
                               ones, uwd_d, wgT_d, gbias_d, out_d)

    nc.compile()
    return nc


def _build_features(nc, tc, xr_d, f3, NT, ident):
    """v[t] = |x[t+1]-x[t]|, ang[t] = arccos(clip(pv*v/((pv+eps)(v+eps)))).

    Layout: rows on partitions (16 tiles of 128), t on free (127).
    Ends by transposing to [t, row] and DMAing into f3 DRAM [T, 3, R].
    """
    xr = xr_d.ap()  # [R, 2L] flat, contiguous per row

    with (
        tc.tile_pool(name="feat_in", bufs=3) as fin,
        tc.tile_pool(name="feat_tmp", bufs=3) as ftmp,
        tc.tile_pool(name="feat_keep", bufs=1) as fkeep,
        tc.tile_pool(name="feat_ps", bufs=3, space="PSUM") as fps,
    ):
        v_all = fkeep.tile([128, NT * T], F32, tag="v_all")
        ang = fkeep.tile([128, NT * T], F32, tag="ang")

        for p in range(NT):
            xt = fin.tile([128, 2 * L], F32, tag="xt")
            nc.sync.dma_start(xt[:], xr[p * 128:(p + 1) * 128, :])
            xl = xt[:].rearrange("p (l c) -> p l c", c=2)
            dx = ftmp.tile([128, T], F32, tag="dx")
            dy = ftmp.tile([128, T], F32, tag="dy")
            nc.vector.tensor_tensor(dx[:], xl[:, 1:, 0], xl[:, :-1, 0],
                                    ALU.subtract)
            nc.vector.tensor_tensor(dy[:], xl[:, 1:, 1], xl[:, :-1, 1],
                                    ALU.subtract)
            ss = ftmp.tile([128, T], F32, tag="ss")
            nc.vector.tensor_tensor(ss[:], dx[:], dx[:], ALU.mult)
            dy2 = ftmp.tile([128, T], F32, tag="dy2")
            nc.vector.tensor_tensor(dy2[:], dy[:], dy[:], ALU.mult)
            nc.vector.tensor_tensor(ss[:], ss[:], dy2[:], ALU.add)
            nc.scalar.activation(v_all[:, p * T:(p + 1) * T], ss[:], AF.Sqrt)

        v3 = v_all[:].rearrange("p (q t) -> p q t", t=T)
        # pv = previous-step speed (first step repeats itself)
        pv = fkeep.tile([128, NT * T], F32, tag="pv")
        pv3 = pv[:].rearrange("p (q t) -> p q t", t=T)
        nc.vector.tensor_copy(pv3[:, :, 1:], v3[:, :, :-1])
        nc.vector.tensor_copy(pv3[:, :, 0:1], v3[:, :, 0:1])

        b1 = fkeep.tile([128, NT * T], F32, tag="b1")
        nc.vector.tensor_scalar_add(b1[:], v_all[:], EPS)
        a1 = fkeep.tile([128, NT * T], F32, tag="a1")
        nc.vector.tensor_scalar_add(a1[:], pv[:], EPS)
        den = fkeep.tile([128, NT * T], F32, tag="den")
        nc.vector.tensor_tensor(den[:], a1[:], b1[:], ALU.mult)
        rden = fkeep.tile([128, NT * T], F32, tag="rden")
        nc.vector.reciprocal(rden[:], den[:])
        cos = fkeep.tile([128, NT * T], F32, tag="cos")
        nc.vector.tensor_tensor(cos[:], pv[:], v_all[:], ALU.mult)
        nc.vector.tensor_tensor(cos[:], cos[:], rden[:], ALU.mult)
        nc.vector.tensor_scalar_min(cos[:], cos[:], 1.0)

        # ang = sqrt(1-cos) * ((AC3*cos + AC2)*cos + AC1)*cos + AC0)
        s1 = fkeep.tile([128, NT * T], F32, tag="s1")
        nc.scalar.activation(s1[:], cos[:], AF.Sqrt, bias=1.0, scale=-1.0)
        poly = fkeep.tile([128, NT * T], F32, tag="poly")
        nc.vector.tensor_scalar(poly[:], cos[:], AC3, AC2, ALU.mult, ALU.add)
        nc.vector.tensor_tensor(poly[:], poly[:], cos[:], ALU.mult)
        nc.vector.tensor_scalar_add(poly[:], poly[:], AC1)
        nc.vector.tensor_tensor(poly[:], poly[:], cos[:], ALU.mult)
        nc.vector.tensor_scalar_add(poly[:], poly[:], AC0)
        nc.vector.tensor_tensor(ang[:], poly[:], s1[:], ALU.mult)

        # Cast to bf16, transpose tile-by-tile to [t, row], DMA into f3.
        vbf = fkeep.tile([128, NT * T], BF16, tag="vbf")
        abf = fkeep.tile([128, NT * T], BF16, tag="abf")
        nc.vector.tensor_copy(vbf[:], v_all[:])
        nc.vector.tensor_copy(abf[:], ang[:])
        onesb = fkeep.tile([128, R], BF16, tag="onesb")
        nc.vector.memset(onesb[:], 1.0)

        vt = fkeep.tile([T, R], BF16, tag="vt")
        at = fkeep.tile([T, R], BF16, tag="at")
        for p in range(NT):
            for src, dst in ((vbf, vt), (abf, at)):
                ps = fps.tile([T, 128], BF16, tag="tp")
                nc.tensor.transpose(ps[:], src[:, p * T:(p + 1) * T],
                                    ident[:])
                nc.vector.tensor_copy(dst[:, p * 128:(p + 1) * 128], ps[:])

        nc.sync.dma_start(f3[:, 0, :], vt[:])
        nc.sync.dma_start(f3[:, 1, :], at[:])
        nc.sync.dma_start(f3[:, 2, :], onesb[0:T, :])


def _build_gru_gat(nc, tc, f3, whhT_d, wih_d, bhhn_d, ident, ones, uwd_d,
                   wgT_d, gbias_d, out_d):
    with (
        tc.tile_pool(name="wpool", bufs=1) as wpool,
        tc.tile_pool(name="hpool", bufs=2) as hpool,
    ):
        whhT = wpool.tile([HID, 3 * HID], BF16, tag="whhT")
        nc.sync.dma_start(whhT[:], whhT_d.ap())
        wih = wpool.tile([3, 3 * HID], BF16, tag="wih")
        nc.sync.dma_start(wih[:], wih_d.ap())
        bhhn = wpool.tile([1, HID], BF16, tag="bhhn")
        nc.sync.dma_start(bhhn[:], bhhn_d.ap())

        h_final = _gru(nc, tc, f3, whhT, wih, bhhn, ident, ones, hpool)
        _gat(nc, tc, h_final, uwd_d, wgT_d, gbias_d, ident, ones, out_d)


def _gru(nc, tc, f3, whhT, wih, bhhn, ident, ones, hpool):
    """127 GRU steps over h [128 hid, 2048 rows] bf16, 4 row-streams."""
    with (
        tc.tile_pool(name="fpool", bufs=4) as fpool,
        tc.tile_pool(name="gru_sb", bufs=2 * NSTREAM) as gsb,
        tc.tile_pool(name="ps_rz", bufs=2, space="PSUM") as ps_rz,
        tc.tile_pool(name="ps_n", bufs=2, space="PSUM") as ps_n,
    ):
        h = hpool.tile([HID, R], BF16, tag="h")
        nc.vector.memset(h[:], 0.0)

        for t in range(T):
            ft = fpool.tile([3, R], BF16, tag="ft")
            nc.sync.dma_start(ft[:], f3[t])
            h_new = hpool.tile([HID, R], BF16, tag="h")
            for s in range(NSTREAM):
                sl = slice(s * SC, (s + 1) * SC)
                prz = ps_rz.tile([128, 2 * SC], F32, tag="prz")
                # r preact -> prz[:, :SC], z preact -> prz[:, SC:]
                nc.tensor.matmul(prz[:, 0:SC], whhT[:, 0:128], h[:, sl],
                                 start=True, stop=False)
                nc.tensor.matmul(prz[:, 0:SC], wih[:, 0:128], ft[:, sl],
                                 start=False, stop=True)
                nc.tensor.matmul(prz[:, SC:], whhT[:, 128:256], h[:, sl],
                                 start=True, stop=False)
                nc.tensor.matmul(prz[:, SC:], wih[:, 128:256], ft[:, sl],
                                 start=False, stop=True)
                rz = gsb.tile([128, 2 * SC], BF16, tag="rz")
                nc.scalar.activation(rz[:], prz[:], AF.Sigmoid)

                pn = ps_n.tile([128, 2 * SC], F32, tag="pn")
                # n-gate recurrent part (+bhh_n) -> pn[:, :SC]
                nc.tensor.matmul(pn[:, 0:SC], whhT[:, 256:384], h[:, sl],
                                 start=True, stop=False)
                nc.tensor.matmul(pn[:, 0:SC], bhhn[:], ones[:, 0:SC],
                                 start=False, stop=True)
                # n-gate input part -> pn[:, SC:]
                nc.tensor.matmul(pn[:, SC:], wih[:, 256:384], ft[:, sl],
                                 start=True, stop=False)
                t2 = gsb.tile([128, SC], BF16, tag="t2")
                nc.vector.tensor_tensor(t2[:], rz[:, 0:SC], pn[:, 0:SC],
                                        ALU.mult)
                # accumulate r*gh_n onto the input part, tanh from PSUM
                nc.tensor.matmul(pn[:, SC:], ident[:], t2[:],
                                 start=False, stop=True)
                nn = gsb.tile([128, SC], BF16, tag="nn")
                nc.scalar.activation(nn[:], pn[:, SC:], AF.Tanh)

                d = gsb.tile([128, SC], BF16, tag="d")
                nc.vector.tensor_tensor(d[:], h[:, sl], nn[:], ALU.subtract)
                nc.vector.tensor_tensor(d[:], rz[:, SC:], d[:], ALU.mult)
                nc.vector.tensor_tensor(h_new[:, sl], nn[:], d[:], ALU.add)
            h = h_new
        return h


def _gat(nc, tc, h, uwd_d, wgT_d, gbias_d, ident, ones, out_d):
    """Attention from node 0 over all nodes, per batch of 128 rows."""
    with tc.tile_pool(name="gat_sb", bufs=1) as gsb:
        uwd = gsb.tile([HID, 2 * HEADS], BF16, tag="uwd")
        nc.sync.dma_start(uwd[:], uwd_d.ap())
        wgT = gsb.tile([HID, HEADS * CD], BF16, tag="wgT")
        nc.sync.dma_start(wgT[:], wgT_d.ap())
        gbias = gsb.tile([1, CD], BF16, tag="gbias")
        nc.sync.dma_start(gbias[:], gbias_d.ap())

        e = gsb.tile([HEADS, R], F32, tag="e")
        with tc.tile_pool(name="gat_ps", bufs=1, space="PSUM") as gps:
            # ssd[h, row] = <xh_row, u_h> ; dsd[h, row] = <xh_row, w_h>
            ssd = gps.tile([HEADS, R], F32, tag="ssd")
            dsd = gps.tile([HEADS, R], F32, tag="dsd")
            for c in range(R // 512):
                cs = slice(c * 512, (c + 1) * 512)
                nc.tensor.matmul(ssd[:, cs], uwd[:, 0:HEADS], h[:, cs],
                                 start=True, stop=True)
                nc.tensor.matmul(dsd[:, cs], uwd[:, HEADS:2 * HEADS],
                                 h[:, cs], start=True, stop=True)
            dsb = gsb.tile([HEADS, R], F32, tag="dsb")
            nc.vector.tensor_copy(dsb[:], dsd[:])

            # e[h, b*128+j] = s[h,b*128+j] + d[h, b*128] (attention logits)
            for b in range(BC):
                bs = slice(b * N, (b + 1) * N)
                nc.vector.tensor_scalar_add(e[:, bs], ssd[:, bs],
                                            dsb[:, b * N:b * N + 1])
        lr = gsb.tile([HEADS, R], F32, tag="lr")
        nc.scalar.activation(lr[:], e[:], AF.Lrelu, alpha=NEG_SLOPE)
        p = gsb.tile([HEADS, R], BF16, tag="p")
        nc.scalar.activation(p[:], lr[:], AF.Exp)

        # softmax denominators per (head, batch)
        ssum = gsb.tile([HEADS, BC], F32, tag="ssum")
        nc.vector.tensor_reduce(ssum[:], p[:].rearrange("h (b j) -> h b j",
                                                        j=N), AX.X, ALU.add)
        srec = gsb.tile([HEADS, BC], F32, tag="srec")
        nc.vector.reciprocal(srec[:], ssum[:])
        palpha = gsb.tile([HEADS, R], BF16, tag="palpha")
        for b in range(BC):
            bs = slice(b * N, (b + 1) * N)
            nc.vector.tensor_scalar_mul(palpha[:, bs], p[:, bs],
                                        srec[:, b:b + 1])

        # transpose alpha and h per batch; ctx[f, (b h)] = sum_j hT[j,f]*aT[j,h]
        with tc.tile_pool(name="gat_ps2", bufs=2, space="PSUM") as gps2:
            pt = gsb.tile([128, HEADS * BC], BF16, tag="pt")
            ht = gsb.tile([128, R], BF16, tag="ht")
            ctx = gps2.tile([128, HEADS * BC], F32, tag="ctx")
            for b in range(BC):
                bs = slice(b * N, (b + 1) * N)
                pps = gps2.tile([128, HEADS], BF16, tag="pps")
                nc.tensor.transpose(pps[:], palpha[:, bs],
                                    ident[0:HEADS, 0:HEADS])
                nc.vector.tensor_copy(pt[:, b * HEADS:(b + 1) * HEADS],
                                      pps[:])
                hps = gps2.tile([128, 128], BF16, tag="hps")
                nc.tensor.transpose(hps[:], h[:, bs], ident[:])
                nc.vector.tensor_copy(ht[:, bs], hps[:])
            for b in range(BC):
                bs = slice(b * N, (b + 1) * N)
                nc.tensor.matmul(ctx[:, b * HEADS:(b + 1) * HEADS],
                                 ht[:, bs],
                                 pt[:, b * HEADS:(b + 1) * HEADS],
                                 start=True, stop=True)
            ctxs = gsb.tile([128, HEADS * BC], BF16, tag="ctxs")
            nc.vector.tensor_copy(ctxs[:], ctx[:])

            # out[b, c] = sum_h (W_h/4) ctx_bh + bias
            op = gps2.tile([BC, CD], F32, tag="op")
            ctx4 = ctxs[:].rearrange("f (b h) -> f h b", h=HEADS)
            for hh in range(HEADS):
                nc.tensor.matmul(op[:], ctx4[:, hh, :],
                                 wgT[:, hh * CD:(hh + 1) * CD],
                                 start=(hh == 0), stop=False)
            nc.tensor.matmul(op[:], ones[:, 0:BC], gbias[:], start=False,
                             stop=True)
            osb = gsb.tile([BC, CD], F32, tag="osb")
            nc.vector.tensor_copy(osb[:], op[:])
            nc.sync.dma_start(out_d.ap(), osb[:])


_NC_CACHE = None


def _get_program():
    global _NC_CACHE
    if _NC_CACHE is None:
        _NC_CACHE = _build_program()
    return _NC_CACHE


def kernel(x, gru_wih, gru_whh, gru_bih, gru_bhh, gat_w, gat_att_src,
           gat_att_dst, gat_bias):
    x = np.asarray(x, np.float32)
    gru_wih = np.asarray(gru_wih, np.float32)
    gru_whh = np.asarray(gru_whh, np.float32)
    gru_bih = np.asarray(gru_bih, np.float32)
    gru_bhh = np.asarray(gru_bhh, np.float32)
    gat_w = np.asarray(gat_w, np.float32)
    gat_att_src = np.asarray(gat_att_src, np.float32)
    gat_att_dst = np.asarray(gat_att_dst, np.float32)
    gat_bias = np.asarray(gat_bias, np.float32)

    bf = ml_dtypes.bfloat16

    whhT = np.ascontiguousarray(gru_whh.T).astype(bf)  # [128, 384]
    # ih lhsT rows: wih[:,0], wih[:,1], bias (bih+bhh for r,z; bih for n)
    bias3 = gru_bih + gru_bhh
    bias3 = bias3.copy()
    bias3[2 * HID:] = gru_bih[2 * HID:]
    wih_aug = np.stack([gru_wih[:, 0], gru_wih[:, 1], bias3]).astype(bf)
    bhh_n = gru_bhh[2 * HID:].reshape(1, HID).astype(bf)
    ident = np.eye(128, dtype=np.float32).astype(bf)

    W = gat_w.reshape(HEADS, CD, CD)  # [h, c, f]
    u = np.einsum("hcf,hc->hf", W, gat_att_src)
    w = np.einsum("hcf,hc->hf", W, gat_att_dst)
    uwd = np.ascontiguousarray(np.concatenate([u, w], 0).T).astype(bf)
    # per-head lhsT [f, c] of W_h/HEADS, laid side by side -> [128, 512]
    wgT = np.ascontiguousarray(
        np.concatenate([(W[h] / HEADS).T for h in range(HEADS)], axis=1)
    ).astype(bf)
    gbias = gat_bias.reshape(1, CD).astype(bf)

    shared = dict(whhT=whhT, wih_aug=wih_aug, bhh_n=bhh_n, ident=ident,
                  uwd=uwd, wgT=wgT, gbias=gbias)
    in_maps = []
    for c in range(N_CORES):
        xc = x[c * BC:(c + 1) * BC].reshape(R, 2 * L)
        in_maps.append({"xr": np.ascontiguousarray(xc), **shared})

    nc = _get_program()
    res = run_bass_kernel_spmd(nc, in_maps, list(range(N_CORES)))
    out = np.concatenate([res.results[c]["out"] for c in range(N_CORES)], 0)
    return out.astype(np.float32)
